# revision 22
# baseline (speedup 1.0000x reference)
"""DN4 retrieval-kNN layer as a Trainium2 Bass/Tile kernel.

Reference computation (shapes hardcoded from the problem spec):
  query_feat  [t=4, wq=75, c=640, 10, 10]  -> q normalized over hw axis (per (wq, c))
  support_feat[t=4, ws=25, c=640, 10, 10]  -> s normalized over c axis (per (way, y))
  relation[t, wq, way, x, y] = sum_c qn[t, wq, x, c] * sn[t, way, c, y]   (x=100, y=500)
  score[t, wq, way] = sum_x sum(top3_y(relation))

Sharding: 8 cores = 4 episodes (t) x 2 query-halves. Core 2t handles queries
[0:38), core 2t+1 handles queries [37:75) (38 rows each; query 37 is computed
twice and deduplicated on the host). No cross-device communication.

Device kernel (per core):
  - host prep: inputs pre-transposed to [c, n, x] and cast to bf16 so loads are
    contiguous HWDGE transfers; a 0/1 segment matrix for the per-query row sum
    rides along as a third input.
  - support normalize over c: ACT squares -> ones-matmul partition reduction ->
    reciprocal+sqrt -> outer-product partition broadcast -> in-place DVE scale.
  - query normalize over hw: ACT squares -> DVE strided reduce -> recip/sqrt ->
    in-place DVE broadcast scale.
  - main loop over 30 groups of 128 flattened (query, x) rows: 5 ways x 5
    K-chunk bf16 matmuls accumulate relation tiles [128, 500] in PSUM; DVE max8
    yields top-8 per row; top-3 summed by a tiny strided reduce; a segment-
    matrix matmul accumulates per-query scores in PSUM.
"""

import sys
import numpy as np

sys.path.insert(0, "/opt/trn_rl_repo")

T, WQ, C, HW = 4, 75, 640, 100
WAY, SHOT = 5, 5
NS = WAY * SHOT          # 25 support images per episode
Y = SHOT * HW            # 500 support descriptors per way
YALL = WAY * Y           # 2500
QPC = 38                 # queries per core (overlapping halves of 75)
KC = C // 128            # 5 contraction chunks of 128
NCORES = 8
NK = 3                   # top-k
ROWS = QPC * HW          # 3800 flattened (query, x) relation rows per core
GROUPS = (ROWS + 127) // 128   # 30 row-groups of <=128
BK = 512                 # PSUM bank stride in fp32 elements

_PROGRAM = None


def _build_program(phases=3, loop_reps=0, loop_scope="main"):
    import concourse.tile as tile
    from concourse import bacc, mybir
    from contextlib import ExitStack, nullcontext

    fp32 = mybir.dt.float32
    bf16 = mybir.dt.bfloat16
    AF = mybir.ActivationFunctionType
    AX = mybir.AxisListType

    nc = bacc.Bacc("TRN2", target_bir_lowering=False, debug=False)
    q_in = nc.declare_dram_parameter("q_in", [C, QPC, HW], bf16, isOutput=False)
    s_in = nc.declare_dram_parameter("s_in", [C, NS, HW], bf16, isOutput=False)
    seg_in = nc.declare_dram_parameter("seg_in", [128, GROUPS, QPC], fp32, isOutput=False)
    score_out = nc.declare_dram_parameter("score_out", [QPC, WAY], fp32, isOutput=True)

    with ExitStack() as ctx:
        tc = ctx.enter_context(tile.TileContext(nc))
        const = ctx.enter_context(tc.tile_pool(name="const", bufs=1))
        sbig = ctx.enter_context(tc.tile_pool(name="sbig", bufs=1))
        stage = ctx.enter_context(tc.tile_pool(name="stage", bufs=3))
        small = ctx.enter_context(tc.tile_pool(name="small", bufs=2))
        t8p = ctx.enter_context(tc.tile_pool(name="t8p", bufs=8))
        psp = ctx.enter_context(tc.tile_pool(name="psp", bufs=7, space="PSUM"))
        pssc = ctx.enter_context(tc.tile_pool(name="pssc", bufs=1, space="PSUM"))

        # Constants
        ones_k = const.tile([128, 1], bf16, name="ones_k")
        nc.vector.memset(ones_k[:], 1.0)
        ones_m = const.tile([1, 128], bf16, name="ones_m")
        nc.vector.memset(ones_m[:], 1.0)

        # Per-(row, way) top-3 sums; rows are flattened (query, x), grouped by 128.
        t3all = sbig.tile([128, GROUPS, WAY], fp32, name="t3all")
        seg = sbig.tile([128, GROUPS, QPC], fp32, name="seg")
        nc.sync.dma_start(out=seg[:], in_=seg_in[:])

        # ------------- loads (all fresh tiles; single-wait DMAs) -------------
        sn = []
        qn = []
        for kc in range(KC):
            snk = sbig.tile([128, WAY, Y], bf16, name=f"sn{kc}")
            sn.append(snk)
            nc.sync.dma_start(
                out=snk[:].rearrange("c w (s x) -> c (w s) x", x=HW),
                in_=s_in[kc * 128:(kc + 1) * 128],
            )
            qnk = sbig.tile([128, QPC, HW], bf16, name=f"qn{kc}")
            qn.append(qnk)
            nc.sync.dma_start(out=qnk[:], in_=q_in[kc * 128:(kc + 1) * 128])

        body_cm = (
            tc.For_i(0, loop_reps, 1)
            if (loop_reps and loop_scope == "compute")
            else nullcontext()
        )
        with body_cm:
            # ------------- support: normalize over c -------------
            ss_t = [
                psp.tile([1, BK], fp32, name=f"ss{yc}", tag="rel")
                for yc in range(WAY)
            ] if phases >= 2 else None
            for kc in range(KC):
                if phases >= 2:
                    sq = stage.tile([128, YALL], bf16, name="sq", tag="sq",
                                    padded_shape=[128, QPC * HW])
                    nc.scalar.activation(
                        sq[:], sn[kc][:].rearrange("c w y -> c (w y)"), AF.Square
                    )
                    for yc in range(WAY):
                        nc.tensor.matmul(
                            ss_t[yc][:, 0:Y],
                            lhsT=ones_k[:],
                            rhs=sq[:, yc * Y:(yc + 1) * Y],
                            start=(kc == 0),
                            stop=(kc == KC - 1),
                        )
            rs_bc = []
            if phases >= 2:
                s_recip = small.tile([1, YALL], fp32, name="s_recip", bufs=1)
                for yc in range(WAY):
                    nc.vector.reciprocal(
                        s_recip[:, yc * Y:(yc + 1) * Y], ss_t[yc][:, 0:Y]
                    )
                s_rs = small.tile([1, YALL], bf16, name="s_rs", bufs=1)
                nc.scalar.activation(s_rs[:], s_recip[:], AF.Sqrt)
                # broadcast 1/||s|| across partitions via outer product,
                # then park it in SBUF as bf16 so the scale multiplies run
                # contiguous and hit the DVE 2x mode
                rs_sb = small.tile([128, WAY, Y], bf16, name="rs_sb", bufs=1)
                for yc in range(WAY):
                    rb = psp.tile([128, BK], fp32, name=f"rs_bc{yc}", tag="rel")
                    rs_bc.append(rb)
                    nc.tensor.matmul(
                        rb[:, 0:Y],
                        lhsT=ones_m[:],
                        rhs=s_rs[:, yc * Y:(yc + 1) * Y],
                        start=True,
                        stop=True,
                    )
                    nc.scalar.copy(rs_sb[:, yc], rb[:, 0:Y])
                for kc in range(KC):
                    nc.vector.tensor_mul(
                        sn[kc][:].rearrange("c w y -> c (w y)"),
                        sn[kc][:].rearrange("c w y -> c (w y)"),
                        rs_sb[:].rearrange("c w y -> c (w y)"),
                    )

            # ------------- query: normalize over hw -------------
            for kc in range(KC):
                qnk = qn[kc]
                if phases >= 2:
                    sqq = stage.tile([128, QPC * HW], bf16, name="sq", tag="sq")
                    nc.scalar.activation(
                        sqq[:], qnk[:].rearrange("c q x -> c (q x)"), AF.Square
                    )
                    ssq = small.tile([128, QPC], fp32, name="ssq")
                    nc.vector.reduce_sum(
                        ssq[:],
                        sqq[:].rearrange("c (q x) -> c q x", x=HW),
                        axis=AX.X,
                    )
                    q_recip = small.tile([128, QPC], fp32, name="q_recip")
                    nc.vector.reciprocal(q_recip[:], ssq[:])
                    rq = small.tile([128, QPC], fp32, name="rq")
                    nc.scalar.activation(rq[:], q_recip[:], AF.Sqrt)
                    nc.vector.tensor_mul(
                        qnk[:],
                        qnk[:],
                        rq[:].unsqueeze(2).broadcast_to([128, QPC, HW]),
                    )

            if phases <= 2:
                score_sb = small.tile([QPC, WAY], fp32, name="score_sb")
                nc.vector.tensor_copy(score_sb[:], sn[0][0:QPC, 0, 0:WAY])
                nc.sync.dma_start(out=score_out[:], in_=score_sb[:])

            # ------------- main loop: relation matmuls + top-3 -------------
            if phases >= 3:
                score_ps = pssc.tile([QPC, WAY], fp32, name="score_ps")
                qn_flat = [q_[:].rearrange("c q x -> c (q x)") for q_ in qn]
                loop_cm = (
                    tc.For_i(0, loop_reps, 1)
                    if (loop_reps and loop_scope == "main")
                    else nullcontext()
                )
                with loop_cm:
                    for g in range(GROUPS):
                        m = min(128, ROWS - g * 128)
                        t8q = t8p.tile([128, WAY * 8], fp32, name="t8q")
                        for w in range(WAY):
                            rel = psp.tile([128, Y], fp32, name="rel", tag="rel")
                            for kc in range(KC):
                                nc.tensor.matmul(
                                    rel[0:m],
                                    lhsT=qn_flat[kc][:, g * 128:g * 128 + m],
                                    rhs=sn[kc][:, w],
                                    start=(kc == 0),
                                    stop=(kc == KC - 1),
                                )
                            nc.vector.max(t8q[0:m, w * 8:(w + 1) * 8], rel[0:m])
                        nc.vector.reduce_sum(
                            t3all[0:m, g],
                            t8q[:].rearrange("p (w k) -> p w k", k=8)[0:m, :, 0:NK],
                            axis=AX.X,
                        )
                        nc.tensor.matmul(
                            score_ps[:],
                            lhsT=seg[0:m, g],
                            rhs=t3all[0:m, g],
                            start=(g == 0),
                            stop=(g == GROUPS - 1),
                        )
                score_sb = small.tile([QPC, WAY], fp32, name="score_sb")
                nc.vector.tensor_copy(score_sb[:], score_ps[:])
        if phases >= 3:
            nc.sync.dma_start(out=score_out[:], in_=score_sb[:])

    nc.compile()
    return nc


def _get_program():
    global _PROGRAM
    if _PROGRAM is None:
        _PROGRAM = _build_program()
    return _PROGRAM


def _seg_matrix():
    seg = np.zeros((128, GROUPS, QPC), dtype=np.float32)
    for r in range(ROWS):
        seg[r % 128, r // 128, r // HW] = 1.0
    return seg


def _make_in_maps(qf, sf):
    import ml_dtypes
    bf = ml_dtypes.bfloat16
    seg = _seg_matrix()
    in_maps = []
    for core in range(NCORES):
        t = core // 2
        q0 = 0 if core % 2 == 0 else WQ - QPC  # 0 or 37
        in_maps.append({
            "q_in": np.ascontiguousarray(
                qf[t, q0:q0 + QPC].transpose(1, 0, 2).astype(bf)),
            "s_in": np.ascontiguousarray(
                sf[t].transpose(1, 0, 2).astype(bf)),
            "seg_in": seg,
        })
    return in_maps


def kernel(query_feat, support_feat, way_num, shot_num, query_num, **_):
    from concourse.bass_utils import run_bass_kernel_spmd

    qf = np.asarray(query_feat, dtype=np.float32).reshape(T, WQ, C, HW)
    sf = np.asarray(support_feat, dtype=np.float32).reshape(T, NS, C, HW)
    assert int(way_num) == WAY and int(shot_num) == SHOT

    in_maps = _make_in_maps(qf, sf)
    res = run_bass_kernel_spmd(_get_program(), in_maps, list(range(NCORES))).results

    out = np.empty((T, WQ, WAY), dtype=np.float32)
    for t in range(T):
        lo = res[2 * t]["score_out"]
        hi = res[2 * t + 1]["score_out"]
        out[t, :QPC] = lo
        out[t, QPC:] = hi[QPC - (WQ - QPC):]  # drop the overlapping query row
    return out


# revision 28
# speedup vs baseline: 1.7512x; 1.7512x over previous
"""DN4 retrieval-kNN layer as a Trainium2 Bass/Tile kernel.

Reference computation (shapes hardcoded from the problem spec):
  query_feat  [t=4, wq=75, c=640, 10, 10]  -> q normalized over hw axis (per (wq, c))
  support_feat[t=4, ws=25, c=640, 10, 10]  -> s normalized over c axis (per (way, y))
  relation[t, wq, way, x, y] = sum_c qn[t, wq, x, c] * sn[t, way, c, y]   (x=100, y=500)
  score[t, wq, way] = sum_x sum(top3_y(relation))

Sharding: 8 cores = 4 episodes (t) x 2 query-halves. Core 2t handles queries
[0:38), core 2t+1 handles queries [37:75) (38 rows each; query 37 is computed
twice and deduplicated on the host). No cross-device communication.

Device kernel (per core):
  - host prep: inputs pre-transposed to [c, n, x] and cast to bf16 so loads are
    contiguous HWDGE transfers; a 0/1 segment matrix for the per-query row sum
    rides along as a third input.
  - support normalize over c: ACT squares -> ones-matmul partition reduction ->
    reciprocal+sqrt -> outer-product partition broadcast -> in-place DVE scale.
  - query normalize over hw: ACT squares -> DVE strided reduce -> recip/sqrt ->
    in-place DVE broadcast scale.
  - main loop over 30 groups of 128 flattened (query, x) rows: 5 ways x 5
    K-chunk bf16 matmuls accumulate relation tiles [128, 500] in PSUM; DVE max8
    yields top-8 per row; top-3 summed by a tiny strided reduce; a segment-
    matrix matmul accumulates per-query scores in PSUM.
"""

import sys
import numpy as np

sys.path.insert(0, "/opt/trn_rl_repo")

T, WQ, C, HW = 4, 75, 640, 100
WAY, SHOT = 5, 5
NS = WAY * SHOT          # 25 support images per episode
Y = SHOT * HW            # 500 support descriptors per way
YALL = WAY * Y           # 2500
QPC = 38                 # queries per core (overlapping halves of 75)
KC = C // 128            # 5 contraction chunks of 128
NCORES = 8
NK = 3                   # top-k
ROWS = QPC * HW          # 3800 flattened (query, x) relation rows per core
GROUPS = (ROWS + 127) // 128   # 30 row-groups of <=128
BK = 512                 # PSUM bank stride in fp32 elements

_PROGRAM = None


def _build_program(phases=3, loop_reps=0, loop_scope="main"):
    import concourse.tile as tile
    from concourse import bacc, mybir
    from contextlib import ExitStack, nullcontext

    fp32 = mybir.dt.float32
    bf16 = mybir.dt.bfloat16
    AF = mybir.ActivationFunctionType
    AX = mybir.AxisListType

    nc = bacc.Bacc("TRN2", target_bir_lowering=False, debug=False)
    q_in = nc.declare_dram_parameter("q_in", [C, QPC, HW], bf16, isOutput=False)
    s_in = nc.declare_dram_parameter("s_in", [C, NS, HW], bf16, isOutput=False)
    seg_in = nc.declare_dram_parameter("seg_in", [128, GROUPS, QPC], fp32, isOutput=False)
    score_out = nc.declare_dram_parameter("score_out", [QPC, WAY], fp32, isOutput=True)

    with ExitStack() as ctx:
        tc = ctx.enter_context(tile.TileContext(nc))
        const = ctx.enter_context(tc.tile_pool(name="const", bufs=1))
        sbig = ctx.enter_context(tc.tile_pool(name="sbig", bufs=1))
        stage = ctx.enter_context(tc.tile_pool(name="stage", bufs=3))
        small = ctx.enter_context(tc.tile_pool(name="small", bufs=2))
        t8p = ctx.enter_context(tc.tile_pool(name="t8p", bufs=8))
        psp = ctx.enter_context(tc.tile_pool(name="psp", bufs=7, space="PSUM"))
        pssc = ctx.enter_context(tc.tile_pool(name="pssc", bufs=1, space="PSUM"))

        # Constants
        ones_k = const.tile([128, 1], bf16, name="ones_k")
        nc.vector.memset(ones_k[:], 1.0)
        ones_m = const.tile([1, 128], bf16, name="ones_m")
        nc.vector.memset(ones_m[:], 1.0)

        # Per-(row, way) top-3 sums; rows are flattened (query, x), grouped by 128.
        t3all = sbig.tile([128, GROUPS, WAY], fp32, name="t3all")
        seg = sbig.tile([128, GROUPS, QPC], fp32, name="seg")
        nc.sync.dma_start(out=seg[:], in_=seg_in[:])

        # ------------- loads (all fresh tiles; single-wait DMAs) -------------
        sn = []
        qn = []
        for kc in range(KC):
            snk = sbig.tile([128, WAY, Y], bf16, name=f"sn{kc}")
            sn.append(snk)
            nc.sync.dma_start(
                out=snk[:].rearrange("c w (s x) -> c (w s) x", x=HW),
                in_=s_in[kc * 128:(kc + 1) * 128],
            )
            qnk = sbig.tile([128, QPC, HW], bf16, name=f"qn{kc}")
            qn.append(qnk)
            nc.sync.dma_start(out=qnk[:], in_=q_in[kc * 128:(kc + 1) * 128])

        body_cm = (
            tc.For_i(0, loop_reps, 1)
            if (loop_reps and loop_scope == "compute")
            else nullcontext()
        )
        with body_cm:
            # ------------- support: normalize over c -------------
            ss_t = [
                psp.tile([1, BK], fp32, name=f"ss{yc}", tag="rel")
                for yc in range(WAY)
            ] if phases >= 2 else None
            for kc in range(KC):
                if phases >= 2:
                    sq = stage.tile([128, YALL], bf16, name="sq", tag="sq",
                                    padded_shape=[128, QPC * HW])
                    s_flat = sn[kc][:].rearrange("c w y -> c (w y)")
                    if kc % 2 == 0:
                        nc.scalar.activation(sq[:], s_flat, AF.Square)
                    else:
                        nc.vector.tensor_mul(sq[:], s_flat, s_flat)
                    for yc in range(WAY):
                        nc.tensor.matmul(
                            ss_t[yc][:, 0:Y],
                            lhsT=ones_k[:],
                            rhs=sq[:, yc * Y:(yc + 1) * Y],
                            start=(kc == 0),
                            stop=(kc == KC - 1),
                        )
            rs_bc = []
            if phases >= 2:
                s_recip = small.tile([1, YALL], fp32, name="s_recip", bufs=1)
                for yc in range(WAY):
                    nc.vector.reciprocal(
                        s_recip[:, yc * Y:(yc + 1) * Y], ss_t[yc][:, 0:Y]
                    )
                s_rs = small.tile([1, YALL], bf16, name="s_rs", bufs=1)
                nc.scalar.activation(s_rs[:], s_recip[:], AF.Sqrt)
                # broadcast 1/||s|| across partitions via outer product,
                # then park it in SBUF as bf16 so the scale multiplies run
                # contiguous and hit the DVE 2x mode
                rs_sb = small.tile([128, WAY, Y], bf16, name="rs_sb", bufs=1)
                for yc in range(WAY):
                    rb = psp.tile([128, BK], fp32, name=f"rs_bc{yc}", tag="rel")
                    rs_bc.append(rb)
                    nc.tensor.matmul(
                        rb[:, 0:Y],
                        lhsT=ones_m[:],
                        rhs=s_rs[:, yc * Y:(yc + 1) * Y],
                        start=True,
                        stop=True,
                    )
                    nc.scalar.copy(rs_sb[:, yc], rb[:, 0:Y])
                for kc in range(KC):
                    nc.vector.tensor_mul(
                        sn[kc][:].rearrange("c w y -> c (w y)"),
                        sn[kc][:].rearrange("c w y -> c (w y)"),
                        rs_sb[:].rearrange("c w y -> c (w y)"),
                    )

            # ------------- query: normalize over hw -------------
            for kc in range(KC):
                qnk = qn[kc]
                if phases >= 2:
                    sqq = stage.tile([128, QPC * HW], bf16, name="sq", tag="sq")
                    nc.scalar.activation(
                        sqq[:], qnk[:].rearrange("c q x -> c (q x)"), AF.Square
                    )
                    ssq = small.tile([128, QPC], fp32, name="ssq")
                    nc.vector.reduce_sum(
                        ssq[:],
                        sqq[:].rearrange("c (q x) -> c q x", x=HW),
                        axis=AX.X,
                    )
                    q_recip = small.tile([128, QPC], fp32, name="q_recip")
                    nc.vector.reciprocal(q_recip[:], ssq[:])
                    rq = small.tile([128, QPC], fp32, name="rq")
                    nc.scalar.activation(rq[:], q_recip[:], AF.Sqrt)
                    # gpsimd is idle here; taking the scale off DVE shortens
                    # the serial normalization chain gating the matmul stream
                    mul_eng = nc.gpsimd if kc % 2 else nc.vector
                    mul_eng.tensor_mul(
                        qnk[:],
                        qnk[:],
                        rq[:].unsqueeze(2).broadcast_to([128, QPC, HW]),
                    )

            if phases <= 2:
                score_sb = small.tile([QPC, WAY], fp32, name="score_sb")
                nc.vector.tensor_copy(score_sb[:], sn[0][0:QPC, 0, 0:WAY])
                nc.sync.dma_start(out=score_out[:], in_=score_sb[:])

            # ------------- main loop: relation matmuls + top-3 -------------
            if phases >= 3:
                score_ps = pssc.tile([QPC, WAY], fp32, name="score_ps")
                qn_flat = [q_[:].rearrange("c q x -> c (q x)") for q_ in qn]
                loop_cm = (
                    tc.For_i(0, loop_reps, 1)
                    if (loop_reps and loop_scope == "main")
                    else nullcontext()
                )
                with loop_cm:
                    for g in range(GROUPS):
                        m = min(128, ROWS - g * 128)
                        t8q = t8p.tile([128, WAY * 8], fp32, name="t8q")
                        for w in range(WAY):
                            rel = psp.tile([128, Y], fp32, name="rel", tag="rel")
                            for kc in range(KC):
                                nc.tensor.matmul(
                                    rel[0:m],
                                    lhsT=qn_flat[kc][:, g * 128:g * 128 + m],
                                    rhs=sn[kc][:, w],
                                    start=(kc == 0),
                                    stop=(kc == KC - 1),
                                )
                            nc.vector.max(t8q[0:m, w * 8:(w + 1) * 8], rel[0:m])
                        nc.vector.reduce_sum(
                            t3all[0:m, g],
                            t8q[:].rearrange("p (w k) -> p w k", k=8)[0:m, :, 0:NK],
                            axis=AX.X,
                        )
                    # segment-matrix accumulation after the relation stream:
                    # placed mid-loop it stalls the in-order PE queue on every
                    # group's DVE reduce
                    for g in range(GROUPS):
                        m = min(128, ROWS - g * 128)
                        nc.tensor.matmul(
                            score_ps[:],
                            lhsT=seg[0:m, g],
                            rhs=t3all[0:m, g],
                            start=(g == 0),
                            stop=(g == GROUPS - 1),
                        )
                score_sb = small.tile([QPC, WAY], fp32, name="score_sb")
                nc.vector.tensor_copy(score_sb[:], score_ps[:])
        if phases >= 3:
            nc.sync.dma_start(out=score_out[:], in_=score_sb[:])

    nc.compile()
    return nc


def _get_program():
    global _PROGRAM
    if _PROGRAM is None:
        _PROGRAM = _build_program()
    return _PROGRAM


def _seg_matrix():
    seg = np.zeros((128, GROUPS, QPC), dtype=np.float32)
    for r in range(ROWS):
        seg[r % 128, r // 128, r // HW] = 1.0
    return seg


def _make_in_maps(qf, sf):
    import ml_dtypes
    bf = ml_dtypes.bfloat16
    seg = _seg_matrix()
    in_maps = []
    for core in range(NCORES):
        t = core // 2
        q0 = 0 if core % 2 == 0 else WQ - QPC  # 0 or 37
        in_maps.append({
            "q_in": np.ascontiguousarray(
                qf[t, q0:q0 + QPC].transpose(1, 0, 2).astype(bf)),
            "s_in": np.ascontiguousarray(
                sf[t].transpose(1, 0, 2).astype(bf)),
            "seg_in": seg,
        })
    return in_maps


def kernel(query_feat, support_feat, way_num, shot_num, query_num, **_):
    from concourse.bass_utils import run_bass_kernel_spmd

    qf = np.asarray(query_feat, dtype=np.float32).reshape(T, WQ, C, HW)
    sf = np.asarray(support_feat, dtype=np.float32).reshape(T, NS, C, HW)
    assert int(way_num) == WAY and int(shot_num) == SHOT

    in_maps = _make_in_maps(qf, sf)
    res = run_bass_kernel_spmd(_get_program(), in_maps, list(range(NCORES))).results

    out = np.empty((T, WQ, WAY), dtype=np.float32)
    for t in range(T):
        lo = res[2 * t]["score_out"]
        hi = res[2 * t + 1]["score_out"]
        out[t, :QPC] = lo
        out[t, QPC:] = hi[QPC - (WQ - QPC):]  # drop the overlapping query row
    return out


# revision 30
# speedup vs baseline: 1.8429x; 1.0524x over previous
"""DN4 retrieval-kNN layer as a Trainium2 Bass/Tile kernel.

Reference computation (shapes hardcoded from the problem spec):
  query_feat  [t=4, wq=75, c=640, 10, 10]  -> q normalized over hw axis (per (wq, c))
  support_feat[t=4, ws=25, c=640, 10, 10]  -> s normalized over c axis (per (way, y))
  relation[t, wq, way, x, y] = sum_c qn[t, wq, x, c] * sn[t, way, c, y]   (x=100, y=500)
  score[t, wq, way] = sum_x sum(top3_y(relation))

Sharding: 8 cores = 4 episodes (t) x 2 query-halves. Core 2t handles queries
[0:38), core 2t+1 handles queries [37:75) (38 rows each; query 37 is computed
twice and deduplicated on the host). No cross-device communication.

Device kernel (per core):
  - host prep: inputs pre-transposed to [c, n, x] and cast to bf16 so loads are
    contiguous HWDGE transfers; a 0/1 segment matrix for the per-query row sum
    rides along as a third input.
  - support normalize over c: ACT squares -> ones-matmul partition reduction ->
    reciprocal+sqrt -> outer-product partition broadcast -> in-place DVE scale.
  - query normalize over hw: ACT squares -> DVE strided reduce -> recip/sqrt ->
    in-place DVE broadcast scale.
  - main loop over 30 groups of 128 flattened (query, x) rows: 5 ways x 5
    K-chunk bf16 matmuls accumulate relation tiles [128, 500] in PSUM; DVE max8
    yields top-8 per row; top-3 summed by a tiny strided reduce; a segment-
    matrix matmul accumulates per-query scores in PSUM.
"""

import sys
import numpy as np

sys.path.insert(0, "/opt/trn_rl_repo")

T, WQ, C, HW = 4, 75, 640, 100
WAY, SHOT = 5, 5
NS = WAY * SHOT          # 25 support images per episode
Y = SHOT * HW            # 500 support descriptors per way
YALL = WAY * Y           # 2500
QPC = 38                 # queries per core (overlapping halves of 75)
KC = C // 128            # 5 contraction chunks of 128
NCORES = 8
NK = 3                   # top-k
ROWS = QPC * HW          # 3800 flattened (query, x) relation rows per core
GROUPS = (ROWS + 127) // 128   # 30 row-groups of <=128
BK = 512                 # PSUM bank stride in fp32 elements

_PROGRAM = None


def _build_program(phases=3, loop_reps=0, loop_scope="main"):
    import concourse.tile as tile
    from concourse import bacc, mybir
    from contextlib import ExitStack, nullcontext

    fp32 = mybir.dt.float32
    bf16 = mybir.dt.bfloat16
    AF = mybir.ActivationFunctionType
    AX = mybir.AxisListType

    nc = bacc.Bacc("TRN2", target_bir_lowering=False, debug=False)
    q_in = nc.declare_dram_parameter("q_in", [C, QPC, HW], bf16, isOutput=False)
    s_in = nc.declare_dram_parameter("s_in", [C, NS, HW], bf16, isOutput=False)
    seg_in = nc.declare_dram_parameter("seg_in", [128, GROUPS, QPC], fp32, isOutput=False)
    score_out = nc.declare_dram_parameter("score_out", [QPC, WAY], fp32, isOutput=True)

    with ExitStack() as ctx:
        tc = ctx.enter_context(tile.TileContext(nc))
        const = ctx.enter_context(tc.tile_pool(name="const", bufs=1))
        sbig = ctx.enter_context(tc.tile_pool(name="sbig", bufs=1))
        stage = ctx.enter_context(tc.tile_pool(name="stage", bufs=3))
        small = ctx.enter_context(tc.tile_pool(name="small", bufs=2))
        t8p = ctx.enter_context(tc.tile_pool(name="t8p", bufs=8))
        psp = ctx.enter_context(tc.tile_pool(name="psp", bufs=8, space="PSUM"))

        # Constants
        ones_k = const.tile([128, 1], bf16, name="ones_k")
        nc.vector.memset(ones_k[:], 1.0)
        ones_m = const.tile([1, 128], bf16, name="ones_m")
        nc.vector.memset(ones_m[:], 1.0)

        # Per-(row, way) top-3 sums; rows are flattened (query, x), grouped by 128.
        t3all = sbig.tile([128, GROUPS, WAY], fp32, name="t3all")
        seg = sbig.tile([128, GROUPS, QPC], fp32, name="seg")
        nc.sync.dma_start(out=seg[:], in_=seg_in[:])

        # ------------- loads (all fresh tiles; single-wait DMAs) -------------
        sn = []
        qn = []
        for kc in range(KC):
            snk = sbig.tile([128, WAY, Y], bf16, name=f"sn{kc}")
            sn.append(snk)
            nc.sync.dma_start(
                out=snk[:].rearrange("c w (s x) -> c (w s) x", x=HW),
                in_=s_in[kc * 128:(kc + 1) * 128],
            )
            qnk = sbig.tile([128, QPC, HW], bf16, name=f"qn{kc}")
            qn.append(qnk)
            nc.sync.dma_start(out=qnk[:], in_=q_in[kc * 128:(kc + 1) * 128])

        body_cm = (
            tc.For_i(0, loop_reps, 1)
            if (loop_reps and loop_scope == "compute")
            else nullcontext()
        )
        with body_cm:
            # ------------- support: normalize over c -------------
            ss_t = [
                psp.tile([1, BK], fp32, name=f"ss{yc}", tag="rel")
                for yc in range(WAY)
            ] if phases >= 2 else None
            for kc in range(KC):
                if phases >= 2:
                    sq = stage.tile([128, YALL], bf16, name="sq", tag="sq",
                                    padded_shape=[128, QPC * HW])
                    s_flat = sn[kc][:].rearrange("c w y -> c (w y)")
                    if kc % 2 == 0:
                        nc.scalar.activation(sq[:], s_flat, AF.Square)
                    else:
                        nc.vector.tensor_mul(sq[:], s_flat, s_flat)
                    for yc in range(WAY):
                        nc.tensor.matmul(
                            ss_t[yc][:, 0:Y],
                            lhsT=ones_k[:],
                            rhs=sq[:, yc * Y:(yc + 1) * Y],
                            start=(kc == 0),
                            stop=(kc == KC - 1),
                        )
                    # interleave the query-norm chain for this chunk so both
                    # normalizations share the engines from the start
                    qnk = qn[kc]
                    sqq = stage.tile([128, QPC * HW], bf16, name="sq", tag="sq")
                    nc.scalar.activation(
                        sqq[:], qnk[:].rearrange("c q x -> c (q x)"), AF.Square
                    )
                    ssq = small.tile([128, QPC], fp32, name="ssq")
                    nc.vector.reduce_sum(
                        ssq[:],
                        sqq[:].rearrange("c (q x) -> c q x", x=HW),
                        axis=AX.X,
                    )
                    q_recip = small.tile([128, QPC], fp32, name="q_recip")
                    nc.vector.reciprocal(q_recip[:], ssq[:])
                    rq = small.tile([128, QPC], fp32, name="rq")
                    nc.scalar.activation(rq[:], q_recip[:], AF.Sqrt)
                    mul_eng = nc.gpsimd if kc % 2 else nc.vector
                    mul_eng.tensor_mul(
                        qnk[:],
                        qnk[:],
                        rq[:].unsqueeze(2).broadcast_to([128, QPC, HW]),
                    )
            if phases >= 2:
                # per-way pipeline: way w's sn columns are fully normalized
                # before later ways finish, matching the main loop's w-order
                s_recip = small.tile([1, YALL], fp32, name="s_recip", bufs=1)
                s_rs = small.tile([1, YALL], bf16, name="s_rs", bufs=1)
                rs_sb = small.tile([128, WAY, Y], bf16, name="rs_sb", bufs=1)
                for yc in range(WAY):
                    nc.vector.reciprocal(
                        s_recip[:, yc * Y:(yc + 1) * Y], ss_t[yc][:, 0:Y]
                    )
                    nc.scalar.activation(
                        s_rs[:, yc * Y:(yc + 1) * Y],
                        s_recip[:, yc * Y:(yc + 1) * Y], AF.Sqrt
                    )
                    rb = psp.tile([128, BK], fp32, name=f"rs_bc{yc}", tag="rel")
                    nc.tensor.matmul(
                        rb[:, 0:Y],
                        lhsT=ones_m[:],
                        rhs=s_rs[:, yc * Y:(yc + 1) * Y],
                        start=True,
                        stop=True,
                    )
                    nc.scalar.copy(rs_sb[:, yc], rb[:, 0:Y])
                    for kc in range(KC):
                        nc.vector.tensor_mul(
                            sn[kc][:, yc], sn[kc][:, yc], rs_sb[:, yc]
                        )

            if phases <= 2:
                score_sb = small.tile([QPC, WAY], fp32, name="score_sb")
                nc.vector.tensor_copy(score_sb[:], sn[0][0:QPC, 0, 0:WAY])
                nc.sync.dma_start(out=score_out[:], in_=score_sb[:])

            # ------------- main loop: relation matmuls + top-3 -------------
            if phases >= 3:
                score_ps = psp.tile([QPC, WAY], fp32, name="score_ps", tag="rel")
                qn_flat = [q_[:].rearrange("c q x -> c (q x)") for q_ in qn]
                loop_cm = (
                    tc.For_i(0, loop_reps, 1)
                    if (loop_reps and loop_scope == "main")
                    else nullcontext()
                )
                with loop_cm:
                    for g in range(GROUPS):
                        m = min(128, ROWS - g * 128)
                        t8q = t8p.tile([128, WAY * 8], fp32, name="t8q")
                        for w in range(WAY):
                            rel = psp.tile([128, Y], fp32, name="rel", tag="rel")
                            for kc in range(KC):
                                nc.tensor.matmul(
                                    rel[0:m],
                                    lhsT=qn_flat[kc][:, g * 128:g * 128 + m],
                                    rhs=sn[kc][:, w],
                                    start=(kc == 0),
                                    stop=(kc == KC - 1),
                                )
                            nc.vector.max(t8q[0:m, w * 8:(w + 1) * 8], rel[0:m])
                        nc.vector.reduce_sum(
                            t3all[0:m, g],
                            t8q[:].rearrange("p (w k) -> p w k", k=8)[0:m, :, 0:NK],
                            axis=AX.X,
                        )
                    # segment-matrix accumulation after the relation stream:
                    # placed mid-loop it stalls the in-order PE queue on every
                    # group's DVE reduce
                    for g in range(GROUPS):
                        m = min(128, ROWS - g * 128)
                        nc.tensor.matmul(
                            score_ps[:],
                            lhsT=seg[0:m, g],
                            rhs=t3all[0:m, g],
                            start=(g == 0),
                            stop=(g == GROUPS - 1),
                        )
                score_sb = small.tile([QPC, WAY], fp32, name="score_sb")
                nc.vector.tensor_copy(score_sb[:], score_ps[:])
        if phases >= 3:
            nc.sync.dma_start(out=score_out[:], in_=score_sb[:])

    nc.compile()
    return nc


def _get_program():
    global _PROGRAM
    if _PROGRAM is None:
        _PROGRAM = _build_program()
    return _PROGRAM


def _seg_matrix():
    seg = np.zeros((128, GROUPS, QPC), dtype=np.float32)
    for r in range(ROWS):
        seg[r % 128, r // 128, r // HW] = 1.0
    return seg


def _make_in_maps(qf, sf):
    import ml_dtypes
    bf = ml_dtypes.bfloat16
    seg = _seg_matrix()
    in_maps = []
    for core in range(NCORES):
        t = core // 2
        q0 = 0 if core % 2 == 0 else WQ - QPC  # 0 or 37
        in_maps.append({
            "q_in": np.ascontiguousarray(
                qf[t, q0:q0 + QPC].transpose(1, 0, 2).astype(bf)),
            "s_in": np.ascontiguousarray(
                sf[t].transpose(1, 0, 2).astype(bf)),
            "seg_in": seg,
        })
    return in_maps


def kernel(query_feat, support_feat, way_num, shot_num, query_num, **_):
    from concourse.bass_utils import run_bass_kernel_spmd

    qf = np.asarray(query_feat, dtype=np.float32).reshape(T, WQ, C, HW)
    sf = np.asarray(support_feat, dtype=np.float32).reshape(T, NS, C, HW)
    assert int(way_num) == WAY and int(shot_num) == SHOT

    in_maps = _make_in_maps(qf, sf)
    res = run_bass_kernel_spmd(_get_program(), in_maps, list(range(NCORES))).results

    out = np.empty((T, WQ, WAY), dtype=np.float32)
    for t in range(T):
        lo = res[2 * t]["score_out"]
        hi = res[2 * t + 1]["score_out"]
        out[t, :QPC] = lo
        out[t, QPC:] = hi[QPC - (WQ - QPC):]  # drop the overlapping query row
    return out
